# revision 11
# baseline (speedup 1.0000x reference)
"""BDH parallel attention (chunked linear attention with interleaved RoPE) on 8 TRN2 cores.

Reference computation (B=1, NH=16, T=4096, N=256, D=1024, CHUNK=128):
  QR = rope(Q); KR == QR; V head-broadcast
  out_c = q_c @ state + tril(q_c q_c^T, -1) @ v_c ; state += q_c^T @ v_c

Implementation: head-parallel (2 heads/core), all matmul operands in bf16
(fp32 accumulation in PSUM), output written bf16 and upcast on host.
State recurrence runs at PAIR granularity (256 rows): dq accumulates two
chunks in PSUM before one DVE fold into the bf16 state; the second chunk's
intra-pair contribution comes from the cross-block scores S01 = q_c0 q_c1^T,
computed in the same F=256 matmul that produces S00.

Engine budget per head-pair (PE ~6.3us): DVE rope+masks+folds ~3.7us,
Act psum->sbuf copies ~3.4us, DMA ~36MB/core total.
"""
import math
import os
import numpy as np

B, NH, T, N, D = 1, 16, 4096, 256, 1024
C = 128                  # chunk length == partition count
NCH = T // C             # 32 chunks
NPR = NCH // 2           # 16 pairs
HPC = NH // 8            # heads per core = 2
THETA = 2.0 ** 16
TWO_PI = 2.0 * math.pi

_CACHE = {}
LAST_EXEC_NS = None


def _tables():
    """cos/sin phase tables [T, N] in fp32, replicating the fp32 reference math."""
    t = np.floor(np.arange(N, dtype=np.float32) / np.float32(2.0)) * np.float32(2.0)
    freqs = (np.float32(1.0) / (np.float32(THETA) ** (t / np.float32(N))) / np.float32(TWO_PI)).astype(np.float32)
    pos = np.arange(T, dtype=np.float32)
    phases = pos[:, None] * freqs[None, :]
    ph = np.mod(phases, np.float32(1.0)) * np.float32(TWO_PI)
    cos_t = np.cos(ph).astype(np.float32)
    sin_t = np.sin(ph).astype(np.float32)
    # fold rot()'s sign into the table: qr_e = q_e*cos_e + q_o*(-sin_e)
    sin_signed = sin_t.copy()
    sin_signed[:, 0::2] = -sin_signed[:, 0::2]
    return cos_t, sin_signed


def _build():
    import concourse.bacc as bacc
    import concourse.mybir as mybir
    import concourse.tile as tile

    f32 = mybir.dt.float32
    bf16 = mybir.dt.bfloat16
    f8 = mybir.dt.float8e4
    P = 128

    nc = bacc.Bacc("TRN2", target_bir_lowering=False, debug=False)

    Qd = nc.dram_tensor("Q", [HPC, T, 2, N], bf16, kind="ExternalInput")  # [h,t,(q|qswap),n]
    Vd = nc.dram_tensor("V", [T, D], bf16, kind="ExternalInput")
    V8d = nc.dram_tensor("V8", [T, D], f8, kind="ExternalInput")          # fp8 copy for intra path
    CSd = nc.dram_tensor("CS", [T, 2 * N], bf16, kind="ExternalInput")    # cos | sin-signed
    Od = nc.dram_tensor("O", [HPC, T, D], bf16, kind="ExternalOutput")

    from contextlib import ExitStack
    with ExitStack() as ctx:
        tc = ctx.enter_context(tile.TileContext(nc))
        pool = lambda name, bufs, **kw: ctx.enter_context(tc.tile_pool(name=name, bufs=bufs, **kw))
        constp = pool("const", 1)
        vp = pool("vp", 8)
        v8p_pool = pool("v8p", 4)
        tblp = pool("tbl", 8)
        qp = pool("qp", 8)
        ropep = pool("ropep", 10)
        qrp = pool("qrp", 20)
        qtp = pool("qtp", 4)
        qt8p = pool("qt8p", 4)
        stmp = pool("stmp", 4)
        ostg = pool("ostg", 8)
        st_pools_00 = pool("st0a", 2)
        st_pools_01 = pool("st0b", 2)
        st_pools_10 = pool("st1a", 2)
        st_pools_11 = pool("st1b", 2)
        dqps = pool("dqps", 2, space="PSUM")   # [128,2,512] f32 -> 2 banks each
        ops = pool("ops", 2, space="PSUM")     # [128,512] f32 -> 1 bank each
        trps = pool("trps", 1, space="PSUM")   # [128,2,2,128] bf16 -> 1 bank
        scps = pool("scps", 1, space="PSUM")   # [128,2,128] f32 -> 1 bank
        st_pools = [[st_pools_00, st_pools_01], [st_pools_10, st_pools_11]]

        # constants: identity (bf16, for PE transpose) + strict-upper mask (bf16)
        ones = constp.tile([P, P], f32, tag="ones")
        ident_f = constp.tile([P, P], f32, tag="ident_f")
        ident = constp.tile([P, P], bf16, tag="ident")
        maskT_f = constp.tile([P, P], f32, tag="maskT_f")
        maskT = constp.tile([P, P], bf16, tag="maskT")
        nc.gpsimd.memset(ones[:], 1.0)
        nc.gpsimd.affine_select(
            ident_f[:], ones[:], pattern=[[1, P]],
            compare_op=mybir.AluOpType.is_equal, fill=0.0,
            base=0, channel_multiplier=-1,
        )
        nc.vector.tensor_copy(ident[:], ident_f[:])
        # maskT[k, t] = 1 if k < t (strict upper)
        nc.gpsimd.affine_select(
            maskT_f[:], ones[:], pattern=[[1, P]],
            compare_op=mybir.AluOpType.is_ge, fill=0.0,
            base=-1, channel_multiplier=-1,
        )
        nc.vector.tensor_copy(maskT[:], maskT_f[:])

        st_cur = [[None, None], [None, None]]  # [h][half] -> sbuf [128,1024] bf16

        def emit_load(i):
            """Load one chunk i: v [P,D], cs [P,2,N], qq [P,HPC,2,N] (all bf16)."""
            r0 = i * C
            v = vp.tile([P, D], bf16, tag="v")
            nc.sync.dma_start(v[:], Vd.ap()[r0:r0 + C, :])
            cs = tblp.tile([P, 2, N], bf16, tag="cs")
            nc.sync.dma_start(cs[:], CSd.ap()[r0:r0 + C, :].rearrange("r (a n) -> r a n", a=2))
            qq = qp.tile([P, HPC, 2, N], bf16, tag="qq")
            nc.sync.dma_start(qq[:], Qd.ap()[:, r0:r0 + C, :, :].rearrange("h r a n -> r h a n"))
            return v, cs, qq

        def emit_load_v8(p):
            """fp8 V for pair p in k-tile layout [kk, chunk(j), d] for DoubleRow apply."""
            r0 = 2 * p * C
            v8 = v8p_pool.tile([P, 2, D], f8, tag="v8")
            nc.sync.dma_start(v8[:], V8d.ap()[r0:r0 + 2 * C, :].rearrange("(j kk) d -> kk j d", j=2))
            return v8

        def emit_rope(i, h):
            """qr = q*cos + qswap*sin' for chunk i, head h (GpSimd, bf16)."""
            cs, qq = loads_cs[i], loads_qq[i]
            t1 = ropep.tile([P, N], bf16, tag="t1")
            t2 = ropep.tile([P, N], bf16, tag="t2")
            qr = qrp.tile([P, N], bf16, tag="qr")
            nc.gpsimd.tensor_mul(t1[:], qq[:, h, 0, :], cs[:, 0, :])
            nc.gpsimd.tensor_mul(t2[:], qq[:, h, 1, :], cs[:, 1, :])
            nc.gpsimd.tensor_add(qr[:], t2[:], t1[:])
            return qr

        def emit_transp(p, h):
            """Transpose both chunks of pair p, head h -> qT bf16 (act) + qT8 fp8 (DVE)."""
            trp = trps.tile([P, 2, 2, P], bf16, tag="trp")  # [k][chunk][half][t]
            for j in range(2):
                qr = ropes[(2 * p + j, h)]
                nc.tensor.transpose(trp[:, j, 0, :], qr[:, 0:P], ident[:])
                nc.tensor.transpose(trp[:, j, 1, :], qr[:, P:N], ident[:])
            qT = qtp.tile([P, 2, 2, P], bf16, tag="qT")
            nc.scalar.copy(qT[:], trp[:])
            qT8 = qt8p.tile([P, 2, 2, P], f8, tag="qT8")
            nc.vector.tensor_copy(qT8[:], trp[:])
            return qT, qT8

        def emit_scores(p, h):
            """Scores blocks for pair p via fp8 DoubleRow -> stm [128,3,128] fp8 stationaries.

            stm[:,0,:] = S00 masked (k<t), stm[:,1,:] = S01 (full), stm[:,2,:] = S11 masked.
            """
            qT8 = qT8ds[(p, h)]
            DR = mybir.MatmulPerfMode.DoubleRow
            scs = scps.tile([P, 3, P], f32, tag="scs")
            # qT8[:, c, :, :] is [kk, j=half, t]; DR contracts over (kk, j) = n
            nc.tensor.matmul(scs[:, 0, :], qT8[:, 0, :, :], qT8[:, 0, :, :],
                             start=True, stop=True, perf_mode=DR)
            nc.tensor.matmul(scs[:, 1, :], qT8[:, 0, :, :], qT8[:, 1, :, :],
                             start=True, stop=True, perf_mode=DR)
            nc.tensor.matmul(scs[:, 2, :], qT8[:, 1, :, :], qT8[:, 1, :, :],
                             start=True, stop=True, perf_mode=DR)
            stm = stmp.tile([P, 3, P], f8, tag="stm")
            nc.vector.tensor_tensor(stm[:, 0, :], scs[:, 0, :], maskT[:], mybir.AluOpType.mult)
            nc.vector.tensor_copy(stm[:, 1, :], scs[:, 1, :])
            nc.vector.tensor_tensor(stm[:, 2, :], scs[:, 2, :], maskT[:], mybir.AluOpType.mult)
            return stm

        def emit_heavy(p, h):
            """out rows for both chunks of pair p, head h: intra (fp8 scores@v) + inter (bf16 q@state)."""
            stm, qT = stmds[(p, h)], qTds[(p, h)]
            v8 = loads_v8[p]
            DR = mybir.MatmulPerfMode.DoubleRow
            for j in range(2):  # chunk within pair
                i = 2 * p + j
                r0 = i * C
                for dsl_i in range(2):
                    dsl = slice(dsl_i * 512, (dsl_i + 1) * 512)
                    op = ops.tile([P, 512], f32, tag="op")
                    if j == 0:
                        nc.tensor.matmul(op[:], stm[:, 0, :], v8[:, 0, dsl],
                                         start=True, stop=(p == 0))
                    else:
                        nc.tensor.matmul(op[:], stm[:, 1:3, :], v8[:, :, dsl],
                                         start=True, stop=(p == 0), perf_mode=DR)
                    if p > 0:
                        nc.tensor.matmul(op[:], qT[:, j, 0, :], st_cur[h][0][:, dsl],
                                         start=False, stop=False)
                        nc.tensor.matmul(op[:], qT[:, j, 1, :], st_cur[h][1][:, dsl],
                                         start=False, stop=True)
                    ost = ostg.tile([P, 512], bf16, tag="ost")
                    nc.scalar.copy(ost[:], op[:])
                    nc.sync.dma_start(Od.ap()[h, r0:r0 + C, dsl], ost[:])

        def emit_dq(p, h):
            """State update for pair p: dq = sum over both chunks of q^T v, fold on DVE."""
            st_new = [None, None]
            for half in range(2):
                nsl = slice(half * P, (half + 1) * P)
                dq = dqps.tile([P, D], f32, tag="dq")
                for dsl_i in range(2):
                    dsl = slice(dsl_i * 512, (dsl_i + 1) * 512)
                    nc.tensor.matmul(dq[:, dsl], ropes[(2 * p, h)][:, nsl],
                                     loads_v[2 * p][:, dsl], start=True, stop=False)
                    nc.tensor.matmul(dq[:, dsl], ropes[(2 * p + 1, h)][:, nsl],
                                     loads_v[2 * p + 1][:, dsl], start=False, stop=True)
                stn = st_pools[h][half].tile([P, D], bf16, name=f"st{h}{half}", tag=f"st{h}{half}")
                if p == 0:
                    nc.vector.tensor_copy(stn[:], dq[:])
                else:
                    nc.vector.tensor_tensor(stn[:], dq[:], st_cur[h][half][:],
                                            mybir.AluOpType.add)
                st_new[half] = stn
            for half in range(2):
                st_cur[h][half] = st_new[half]

        loads_v, loads_cs, loads_qq, loads_v8 = {}, {}, {}, {}
        ropes, qTds, qT8ds, stmds = {}, {}, {}, {}

        def do_loads(i):
            v, cs, qq = emit_load(i)
            loads_v[i], loads_cs[i], loads_qq[i] = v, cs, qq

        # prologue: loads for pairs 0-2, rope pairs 0-1, transp+scores pair 0
        for i in range(6):
            do_loads(i)
        for pp in range(3):
            loads_v8[pp] = emit_load_v8(pp)
        for pp in range(2):
            for h in range(HPC):
                for j in range(2):
                    ropes[(2 * pp + j, h)] = emit_rope(2 * pp + j, h)
        for h in range(HPC):
            qTds[(0, h)], qT8ds[(0, h)] = emit_transp(0, h)
            stmds[(0, h)] = emit_scores(0, h)

        for p in range(NPR):
            for h in range(HPC):
                if p + 3 < NPR:
                    do_loads(2 * (p + 3) + h)
                    if h == 0:
                        loads_v8[p + 3] = emit_load_v8(p + 3)
                if p + 2 < NPR:
                    for j in range(2):
                        ropes[(2 * (p + 2) + j, h)] = emit_rope(2 * (p + 2) + j, h)
                if p + 1 < NPR:
                    qTds[(p + 1, h)], qT8ds[(p + 1, h)] = emit_transp(p + 1, h)
                emit_heavy(p, h)
                if p + 1 < NPR:
                    stmds[(p + 1, h)] = emit_scores(p + 1, h)
                if p < NPR - 1:
                    emit_dq(p, h)
            # retire references for pair p
            for j in range(2):
                i = 2 * p + j
                loads_v.pop(i, None); loads_cs.pop(i, None); loads_qq.pop(i, None)
                for h in range(HPC):
                    ropes.pop((i, h), None)
            loads_v8.pop(p, None)
            for h in range(HPC):
                qTds.pop((p, h), None)
                qT8ds.pop((p, h), None)
                stmds.pop((p, h), None)

    nc.compile()
    return nc


def _get_nc():
    if "nc" not in _CACHE:
        _CACHE["nc"] = _build()
    return _CACHE["nc"]


def kernel(**inputs) -> np.ndarray:
    global LAST_EXEC_NS
    import ml_dtypes
    from concourse.bass_utils import run_bass_kernel_spmd

    bf16 = ml_dtypes.bfloat16
    Q_raw = np.ascontiguousarray(np.asarray(inputs["Q_raw"], dtype=np.float32))
    V_raw = np.ascontiguousarray(np.asarray(inputs["V_raw"], dtype=np.float32))

    cos_t, sin_t = _tables()
    cs = np.ascontiguousarray(np.concatenate([cos_t, sin_t], axis=1)).astype(bf16)  # [T, 2N]
    v_b = np.ascontiguousarray(V_raw[0]).astype(bf16)
    v_8 = np.ascontiguousarray(V_raw[0]).astype(ml_dtypes.float8_e4m3fn)

    # QQ[h, t, 0, :] = q ; QQ[h, t, 1, :] = pair-swapped q (sign folded into sin table)
    Q = Q_raw[0]                                  # [NH, T, N]
    Qsw = np.empty_like(Q)
    Qsw[..., 0::2] = Q[..., 1::2]
    Qsw[..., 1::2] = Q[..., 0::2]
    QQ = np.stack([Q, Qsw], axis=2).astype(bf16)  # [NH, T, 2, N]

    nc = _get_nc()
    in_maps = []
    for c in range(8):
        in_maps.append({
            "Q": np.ascontiguousarray(QQ[c * HPC:(c + 1) * HPC]),
            "V": v_b,
            "V8": v_8,
            "CS": cs,
        })

    trace = bool(int(os.environ.get("BDH_TRACE", "0")))
    if trace:
        # NTFF profiling needs the antenv.axon_hooks shim; degrade to
        # no-trace if the ctypes driver is unavailable in this container.
        try:
            import sys as _sys, types as _types
            if "antenv.axon_hooks" not in _sys.modules:
                from trn_agent_boot.trn_boot import _ntff_profile_via_ctypes
                _hook = _ntff_profile_via_ctypes("/opt/axon/libaxon_pjrt.so")
                _mod = _types.ModuleType("antenv.axon_hooks")
                _mod.get_axon_ntff_profile_hook = lambda: _hook
                _sys.modules["antenv.axon_hooks"] = _mod
        except Exception:
            trace = False
    try:
        res = run_bass_kernel_spmd(nc, in_maps, core_ids=list(range(8)), trace=trace)
    except ModuleNotFoundError:
        res = run_bass_kernel_spmd(nc, in_maps, core_ids=list(range(8)), trace=False)
    LAST_EXEC_NS = res.exec_time_ns

    out = np.empty((B, NH, T, D), dtype=np.float32)
    for c in range(8):
        out[0, c * HPC:(c + 1) * HPC] = res.results[c]["O"].astype(np.float32)
    return out


# revision 26
# speedup vs baseline: 1.1718x; 1.1718x over previous
"""BDH parallel attention (chunked linear attention with interleaved RoPE) on 8 TRN2 cores.

Reference computation (B=1, NH=16, T=4096, N=256, D=1024, CHUNK=128):
  QR = rope(Q); KR == QR; V head-broadcast
  out_c = q_c @ state + tril(q_c q_c^T, -1) @ v_c ; state += q_c^T @ v_c

Implementation: head-parallel (2 heads/core). Inter/state path (q@state,
q^T v) runs in bf16 operands with fp32 PSUM accumulation; the intra path
(scores + scores@v) runs in fp8e4 with DoubleRow matmuls (2 fp8 MACs per
PE cell per cycle) - its error is diluted because the intra term's sigma
(~180) is small against the output absmax (~5757). Output written bf16,
upcast on host.

State recurrence runs at PAIR granularity (256 rows): dq accumulates two
chunks in PSUM before one DVE fold into the bf16 state; the second chunk's
intra-pair contribution comes from the cross-block scores S01 = q_c0 q_c1^T.
DoubleRow operand layout is [K=128, 2, free] - the natural k-tile layouts
of qT ([kk, half, t]), stm ([kk, chunk, t]) and the pair-interleaved V8
([kk, chunk, d]), so no extra interleave copies are needed.

Measured on TRN2: ~199-201us HW exec (vs 230us fp32r baseline), rel err
1.35e-2 (absmax-normalized max, gate 2e-2). PE streams at 2.4GHz when the
mix includes fp8 (the all-bf16 variant drew enough power to downclock to
2.0GHz). Engine budget per head-pair-iteration (~6.2us): PE ~4.9us busy,
GpSimd rope ~4us, DVE masks+qT8+folds ~4.1us, Act copies ~3.4us, single
sync-queue DMA ~5.3us (42MB/core total). NOTE: the schedule is a sharp
local optimum - deeper pools, DMA queue splits (scalar/gpsimd-triggered
DMA), cross-engine rope splits, and emission reorders all measured WORSE.
"""
import math
import os
import numpy as np

B, NH, T, N, D = 1, 16, 4096, 256, 1024
C = 128                  # chunk length == partition count
NCH = T // C             # 32 chunks
NPR = NCH // 2           # 16 pairs
HPC = NH // 8            # heads per core = 2
THETA = 2.0 ** 16
TWO_PI = 2.0 * math.pi

_CACHE = {}
LAST_EXEC_NS = None


def _tables():
    """cos/sin phase tables [T, N] in fp32, replicating the fp32 reference math."""
    t = np.floor(np.arange(N, dtype=np.float32) / np.float32(2.0)) * np.float32(2.0)
    freqs = (np.float32(1.0) / (np.float32(THETA) ** (t / np.float32(N))) / np.float32(TWO_PI)).astype(np.float32)
    pos = np.arange(T, dtype=np.float32)
    phases = pos[:, None] * freqs[None, :]
    ph = np.mod(phases, np.float32(1.0)) * np.float32(TWO_PI)
    cos_t = np.cos(ph).astype(np.float32)
    sin_t = np.sin(ph).astype(np.float32)
    # fold rot()'s sign into the table: qr_e = q_e*cos_e + q_o*(-sin_e)
    sin_signed = sin_t.copy()
    sin_signed[:, 0::2] = -sin_signed[:, 0::2]
    return cos_t, sin_signed


def _build():
    import concourse.bacc as bacc
    import concourse.mybir as mybir
    import concourse.tile as tile

    f32 = mybir.dt.float32
    bf16 = mybir.dt.bfloat16
    f8 = mybir.dt.float8e4
    P = 128

    nc = bacc.Bacc("TRN2", target_bir_lowering=False, debug=False)

    Qd = nc.dram_tensor("Q", [HPC, T, 2, N], bf16, kind="ExternalInput")  # [h,t,(q|qswap),n]
    Vd = nc.dram_tensor("V", [T, D], bf16, kind="ExternalInput")
    V8d = nc.dram_tensor("V8", [T, D], f8, kind="ExternalInput")          # fp8 copy for intra path
    CSd = nc.dram_tensor("CS", [T, 2 * N], bf16, kind="ExternalInput")    # cos | sin-signed
    Od = nc.dram_tensor("O", [HPC, T, D], bf16, kind="ExternalOutput")

    from contextlib import ExitStack
    with ExitStack() as ctx:
        tc = ctx.enter_context(tile.TileContext(nc))
        pool = lambda name, bufs, **kw: ctx.enter_context(tc.tile_pool(name=name, bufs=bufs, **kw))
        constp = pool("const", 1)
        vp = pool("vp", 8)
        v8p_pool = pool("v8p", 4)
        tblp = pool("tbl", 6)
        qp = pool("qp", 6)
        ropep = pool("ropep", 6)
        qrp = pool("qrp", 12)
        qtp = pool("qtp", 4)
        qt8p = pool("qt8p", 4)
        stmp = pool("stmp", 4)
        ostg = pool("ostg", 8)
        st_pools_00 = pool("st0a", 2)
        st_pools_01 = pool("st0b", 2)
        st_pools_10 = pool("st1a", 2)
        st_pools_11 = pool("st1b", 2)
        dqps = pool("dqps", 2, space="PSUM")   # [128,2,512] f32 -> 2 banks each
        ops = pool("ops", 2, space="PSUM")     # [128,512] f32 -> 1 bank each
        trps = pool("trps", 1, space="PSUM")   # [128,2,2,128] bf16 -> 1 bank
        scps = pool("scps", 1, space="PSUM")   # [128,2,128] f32 -> 1 bank
        st_pools = [[st_pools_00, st_pools_01], [st_pools_10, st_pools_11]]

        # constants: identity (bf16, for PE transpose) + strict-upper mask (bf16)
        ones = constp.tile([P, P], f32, tag="ones")
        ident_f = constp.tile([P, P], f32, tag="ident_f")
        ident = constp.tile([P, P], bf16, tag="ident")
        maskT_f = constp.tile([P, P], f32, tag="maskT_f")
        maskT = constp.tile([P, P], bf16, tag="maskT")
        nc.gpsimd.memset(ones[:], 1.0)
        nc.gpsimd.affine_select(
            ident_f[:], ones[:], pattern=[[1, P]],
            compare_op=mybir.AluOpType.is_equal, fill=0.0,
            base=0, channel_multiplier=-1,
        )
        nc.vector.tensor_copy(ident[:], ident_f[:])
        # maskT[k, t] = 1 if k < t (strict upper)
        nc.gpsimd.affine_select(
            maskT_f[:], ones[:], pattern=[[1, P]],
            compare_op=mybir.AluOpType.is_ge, fill=0.0,
            base=-1, channel_multiplier=-1,
        )
        nc.vector.tensor_copy(maskT[:], maskT_f[:])

        st_cur = [[None, None], [None, None]]  # [h][half] -> sbuf [128,1024] bf16

        def emit_load(i):
            """Load one chunk i: v [P,D], cs [P,2,N], qq [P,HPC,2,N] (all bf16)."""
            r0 = i * C
            v = vp.tile([P, D], bf16, tag="v")
            nc.sync.dma_start(v[:], Vd.ap()[r0:r0 + C, :])
            cs = tblp.tile([P, 2, N], bf16, tag="cs")
            nc.sync.dma_start(cs[:], CSd.ap()[r0:r0 + C, :].rearrange("r (a n) -> r a n", a=2))
            qq = qp.tile([P, HPC, 2, N], bf16, tag="qq")
            nc.sync.dma_start(qq[:], Qd.ap()[:, r0:r0 + C, :, :].rearrange("h r a n -> r h a n"))
            return v, cs, qq

        def emit_load_v8(p):
            """fp8 V for pair p in k-tile layout [kk, chunk(j), d] for DoubleRow apply."""
            r0 = 2 * p * C
            v8 = v8p_pool.tile([P, 2, D], f8, tag="v8")
            nc.sync.dma_start(v8[:], V8d.ap()[r0:r0 + 2 * C, :].rearrange("(j kk) d -> kk j d", j=2))
            return v8

        def emit_rope(i, h):
            """qr = q*cos + qswap*sin' for chunk i, head h (GpSimd, bf16)."""
            cs, qq = loads_cs[i], loads_qq[i]
            t1 = ropep.tile([P, N], bf16, tag="t1")
            t2 = ropep.tile([P, N], bf16, tag="t2")
            qr = qrp.tile([P, N], bf16, tag="qr")
            nc.gpsimd.tensor_mul(t1[:], qq[:, h, 0, :], cs[:, 0, :])
            nc.gpsimd.tensor_mul(t2[:], qq[:, h, 1, :], cs[:, 1, :])
            nc.gpsimd.tensor_add(qr[:], t2[:], t1[:])
            return qr

        def emit_transp(p, h):
            """Transpose both chunks of pair p, head h -> qT bf16 (act) + qT8 fp8 (DVE)."""
            trp = trps.tile([P, 2, 2, P], bf16, tag="trp")  # [k][chunk][half][t]
            for j in range(2):
                qr = ropes[(2 * p + j, h)]
                nc.tensor.transpose(trp[:, j, 0, :], qr[:, 0:P], ident[:])
                nc.tensor.transpose(trp[:, j, 1, :], qr[:, P:N], ident[:])
            qT = qtp.tile([P, 2, 2, P], bf16, tag="qT")
            nc.scalar.copy(qT[:], trp[:])
            qT8 = qt8p.tile([P, 2, 2, P], f8, tag="qT8")
            nc.vector.tensor_copy(qT8[:], trp[:])
            return qT, qT8

        def emit_scores(p, h):
            """Scores blocks for pair p via fp8 DoubleRow -> stm [128,3,128] fp8 stationaries.

            stm[:,0,:] = S00 masked (k<t), stm[:,1,:] = S01 (full), stm[:,2,:] = S11 masked.
            """
            qT8 = qT8ds[(p, h)]
            DR = mybir.MatmulPerfMode.DoubleRow
            scs = scps.tile([P, 3, P], f32, tag="scs")
            # qT8[:, c, :, :] is [kk, j=half, t]; DR contracts over (kk, j) = n
            nc.tensor.matmul(scs[:, 0, :], qT8[:, 0, :, :], qT8[:, 0, :, :],
                             start=True, stop=True, perf_mode=DR)
            nc.tensor.matmul(scs[:, 1, :], qT8[:, 0, :, :], qT8[:, 1, :, :],
                             start=True, stop=True, perf_mode=DR)
            nc.tensor.matmul(scs[:, 2, :], qT8[:, 1, :, :], qT8[:, 1, :, :],
                             start=True, stop=True, perf_mode=DR)
            stm = stmp.tile([P, 3, P], f8, tag="stm")
            nc.vector.tensor_tensor(stm[:, 0, :], scs[:, 0, :], maskT[:], mybir.AluOpType.mult)
            nc.vector.tensor_copy(stm[:, 1, :], scs[:, 1, :])
            nc.vector.tensor_tensor(stm[:, 2, :], scs[:, 2, :], maskT[:], mybir.AluOpType.mult)
            return stm

        def emit_heavy(p, h):
            """out rows for both chunks of pair p, head h: intra (fp8 scores@v) + inter (bf16 q@state)."""
            stm, qT = stmds[(p, h)], qTds[(p, h)]
            v8 = loads_v8[p]
            DR = mybir.MatmulPerfMode.DoubleRow
            for j in range(2):  # chunk within pair
                i = 2 * p + j
                r0 = i * C
                for dsl_i in range(2):
                    dsl = slice(dsl_i * 512, (dsl_i + 1) * 512)
                    op = ops.tile([P, 512], f32, tag="op")
                    if j == 0:
                        nc.tensor.matmul(op[:], stm[:, 0, :], v8[:, 0, dsl],
                                         start=True, stop=(p == 0))
                    else:
                        nc.tensor.matmul(op[:], stm[:, 1:3, :], v8[:, :, dsl],
                                         start=True, stop=(p == 0), perf_mode=DR)
                    if p > 0:
                        nc.tensor.matmul(op[:], qT[:, j, 0, :], st_cur[h][0][:, dsl],
                                         start=False, stop=False)
                        nc.tensor.matmul(op[:], qT[:, j, 1, :], st_cur[h][1][:, dsl],
                                         start=False, stop=True)
                    ost = ostg.tile([P, 512], bf16, tag="ost")
                    nc.scalar.copy(ost[:], op[:])
                    nc.sync.dma_start(Od.ap()[h, r0:r0 + C, dsl], ost[:])

        def emit_dq(p, h):
            """State update for pair p: dq = sum over both chunks of q^T v, fold on DVE."""
            st_new = [None, None]
            for half in range(2):
                nsl = slice(half * P, (half + 1) * P)
                dq = dqps.tile([P, D], f32, tag="dq")
                for dsl_i in range(2):
                    dsl = slice(dsl_i * 512, (dsl_i + 1) * 512)
                    nc.tensor.matmul(dq[:, dsl], ropes[(2 * p, h)][:, nsl],
                                     loads_v[2 * p][:, dsl], start=True, stop=False)
                    nc.tensor.matmul(dq[:, dsl], ropes[(2 * p + 1, h)][:, nsl],
                                     loads_v[2 * p + 1][:, dsl], start=False, stop=True)
                stn = st_pools[h][half].tile([P, D], bf16, name=f"st{h}{half}", tag=f"st{h}{half}")
                if p == 0:
                    nc.vector.tensor_copy(stn[:], dq[:])
                else:
                    nc.vector.tensor_tensor(stn[:], dq[:], st_cur[h][half][:],
                                            mybir.AluOpType.add)
                st_new[half] = stn
            for half in range(2):
                st_cur[h][half] = st_new[half]

        loads_v, loads_cs, loads_qq, loads_v8 = {}, {}, {}, {}
        ropes, qTds, qT8ds, stmds = {}, {}, {}, {}

        def do_loads(i):
            v, cs, qq = emit_load(i)
            loads_v[i], loads_cs[i], loads_qq[i] = v, cs, qq

        # prologue: loads for pairs 0-2, rope pairs 0-1, transp+scores pair 0
        for i in range(6):
            do_loads(i)
        for pp in range(3):
            loads_v8[pp] = emit_load_v8(pp)
        for pp in range(2):
            for h in range(HPC):
                for j in range(2):
                    ropes[(2 * pp + j, h)] = emit_rope(2 * pp + j, h)
        for h in range(HPC):
            qTds[(0, h)], qT8ds[(0, h)] = emit_transp(0, h)
            stmds[(0, h)] = emit_scores(0, h)

        for p in range(NPR):
            for h in range(HPC):
                if p + 3 < NPR:
                    do_loads(2 * (p + 3) + h)
                    if h == 0:
                        loads_v8[p + 3] = emit_load_v8(p + 3)
                if p + 2 < NPR:
                    for j in range(2):
                        ropes[(2 * (p + 2) + j, h)] = emit_rope(2 * (p + 2) + j, h)
                if p + 1 < NPR:
                    qTds[(p + 1, h)], qT8ds[(p + 1, h)] = emit_transp(p + 1, h)
                emit_heavy(p, h)
                if p + 1 < NPR:
                    stmds[(p + 1, h)] = emit_scores(p + 1, h)
                if p < NPR - 1:
                    emit_dq(p, h)
            # retire references for pair p
            for j in range(2):
                i = 2 * p + j
                loads_v.pop(i, None); loads_cs.pop(i, None); loads_qq.pop(i, None)
                for h in range(HPC):
                    ropes.pop((i, h), None)
            loads_v8.pop(p, None)
            for h in range(HPC):
                qTds.pop((p, h), None)
                qT8ds.pop((p, h), None)
                stmds.pop((p, h), None)

    nc.compile()
    return nc


def _get_nc():
    if "nc" not in _CACHE:
        _CACHE["nc"] = _build()
    return _CACHE["nc"]


def kernel(**inputs) -> np.ndarray:
    global LAST_EXEC_NS
    import ml_dtypes
    from concourse.bass_utils import run_bass_kernel_spmd

    bf16 = ml_dtypes.bfloat16
    Q_raw = np.ascontiguousarray(np.asarray(inputs["Q_raw"], dtype=np.float32))
    V_raw = np.ascontiguousarray(np.asarray(inputs["V_raw"], dtype=np.float32))

    cos_t, sin_t = _tables()
    cs = np.ascontiguousarray(np.concatenate([cos_t, sin_t], axis=1)).astype(bf16)  # [T, 2N]
    v_b = np.ascontiguousarray(V_raw[0]).astype(bf16)
    v_8 = np.ascontiguousarray(V_raw[0]).astype(ml_dtypes.float8_e4m3fn)

    # QQ[h, t, 0, :] = q ; QQ[h, t, 1, :] = pair-swapped q (sign folded into sin table)
    Q = Q_raw[0]                                  # [NH, T, N]
    Qsw = np.empty_like(Q)
    Qsw[..., 0::2] = Q[..., 1::2]
    Qsw[..., 1::2] = Q[..., 0::2]
    QQ = np.stack([Q, Qsw], axis=2).astype(bf16)  # [NH, T, 2, N]

    nc = _get_nc()
    in_maps = []
    for c in range(8):
        in_maps.append({
            "Q": np.ascontiguousarray(QQ[c * HPC:(c + 1) * HPC]),
            "V": v_b,
            "V8": v_8,
            "CS": cs,
        })

    trace = bool(int(os.environ.get("BDH_TRACE", "0")))
    if trace:
        # NTFF profiling needs the antenv.axon_hooks shim; degrade to
        # no-trace if the ctypes driver is unavailable in this container.
        try:
            import sys as _sys, types as _types
            if "antenv.axon_hooks" not in _sys.modules:
                from trn_agent_boot.trn_boot import _ntff_profile_via_ctypes
                _hook = _ntff_profile_via_ctypes("/opt/axon/libaxon_pjrt.so")
                _mod = _types.ModuleType("antenv.axon_hooks")
                _mod.get_axon_ntff_profile_hook = lambda: _hook
                _sys.modules["antenv.axon_hooks"] = _mod
        except Exception:
            trace = False
    try:
        res = run_bass_kernel_spmd(nc, in_maps, core_ids=list(range(8)), trace=trace)
    except ModuleNotFoundError:
        res = run_bass_kernel_spmd(nc, in_maps, core_ids=list(range(8)), trace=False)
    LAST_EXEC_NS = res.exec_time_ns

    out = np.empty((B, NH, T, D), dtype=np.float32)
    for c in range(8):
        out[0, c * HPC:(c + 1) * HPC] = res.results[c]["O"].astype(np.float32)
    return out
